# revision 18
# baseline (speedup 1.0000x reference)
"""Trainium2 Bass kernel: pointnet-style conv stack + score head + top/bottom-K
selection + tiny classifier.

Pipeline (per batch b of 4):
  xT = x[b].T                      [2048, 20000]
  h  = relu(bn(conv 2048->32->8->32))   (conv1d k=1 == matmul over channels)
  s  = relu(bn(conv 32->1))        scores [20000]
  sel = bottom-10 + top-10 indices of stable-ascending argsort(s)
  feat = [s[sel], mean(h[:, sel], -1), h[:, sel].flat]  (692)
  out[b] = sigmoid(classifier(feat))

Strategy:
  * 8 cores = 4 batches x 2 N-halves; each core gets an x.T shard
    [2048, 10000] (host-transposed so the contraction dim D lands on SBUF
    partitions).  The kernel is memory-bound on reading x (82 MB/core).
  * The device computes APPROXIMATE scores s for every column: layer 1
    runs in fp8e4m3 (host-cast x; quarter DMA bytes) with DoubleRow
    matmuls (K=256 per pass), the tiny later layers in fp16.  The
    small layers are software-pipelined 1-3 subblocks behind layer 1
    so the PE never stalls on the activation chain.
  * The host then takes a provably-safe candidate band around the
    bottom-10/top-10 of the approximate scores, recomputes those few
    columns exactly in fp32, and does the exact selection + tiny
    classifier.  Final output error does not depend on device precision
    as long as the band covers the device's score error (band width is
    validated against the observed error and widened if needed).
"""

import numpy as np

import concourse.bass as bass
import concourse.mybir as mybir
import concourse.tile as tile
from concourse.bass_utils import run_bass_kernel_spmd

F32 = mybir.dt.float32
F16 = mybir.dt.float16
F8 = mybir.dt.float8e4

B = 4
N = 20000
D = 2048
H1 = 32
H2 = 8
K = 10
EPS = 1e-5
NCORES = 8
NSH = N // 2           # 10000 columns per core shard
BLOCK = 2000           # columns of x per DMA (4.2 MB fp8 per transfer)
SUB = 500              # matmul moving free dim (<= 512 for 4-byte dtypes)
NCH = D // 128         # 16 contraction chunks of 128

_CACHE = {}


def _split_multi_waits(nc):
    """Walrus in this container only encodes ONE sync wait per instruction
    ("Too many sync wait commands").  Tile attaches several (PE sem + DMA
    lane sems...).  Hoist all-but-one wait onto standalone InstEventSemaphore
    instructions on the same engine queue right before the instruction —
    engine queues are in-order, so semantics are preserved."""
    wid = 0
    for f in nc.m.functions:
        for blk in f.blocks:
            insts = blk.instructions
            for idx in range(len(insts) - 1, -1, -1):
                inst = insts[idx]
                si = inst.sync_info
                if si is None or len(si.on_wait) <= 1:
                    continue
                waits = list(si.on_wait)
                inst.sync_info = mybir.SyncInfo(
                    on_wait=[waits[-1]], on_update=list(si.on_update)
                )
                for w in reversed(waits[:-1]):
                    wid += 1
                    ev = mybir.InstEventSemaphore(
                        name=f"WSPLIT-{wid}", ins=[], outs=[]
                    )
                    ev.engine = inst.engine
                    ev.sync_info = mybir.SyncInfo(on_wait=[w], on_update=[])
                    insts.insert(idx, ev)


def _build_nc():
    nc = bass.Bass()
    xt = nc.declare_dram_parameter("xt", [D, NSH], F8, isOutput=False)
    w1 = nc.declare_dram_parameter("w1", [128, NCH, H1], F8, isOutput=False)
    w2 = nc.declare_dram_parameter("w2", [H1, H2], F16, isOutput=False)
    w3 = nc.declare_dram_parameter("w3", [H2, H1], F16, isOutput=False)
    ws = nc.declare_dram_parameter("ws", [H1, 1], F16, isOutput=False)
    b1 = nc.declare_dram_parameter("b1", [H1, 1], F32, isOutput=False)
    b2 = nc.declare_dram_parameter("b2", [H2, 1], F32, isOutput=False)
    b3 = nc.declare_dram_parameter("b3", [H1, 1], F32, isOutput=False)
    bs = nc.declare_dram_parameter("bs", [1, 1], F32, isOutput=False)
    so = nc.declare_dram_parameter("s", [1, NSH], F32, isOutput=True)

    relu = mybir.ActivationFunctionType.Relu

    JTOT = NSH // SUB          # 20 subblocks
    SPB = BLOCK // SUB         # subblocks per x DMA block

    with tile.TileContext(nc) as tc:
        with (
            tc.tile_pool(name="consts", bufs=1) as consts,
            tc.tile_pool(name="xpool", bufs=2) as xpool,
            tc.tile_pool(name="hpool", bufs=3) as hpool,
            tc.tile_pool(name="opool", bufs=1) as opool,
            tc.tile_pool(name="pspool", bufs=2, space="PSUM") as pspool,
        ):
            w1sb = consts.tile([128, NCH, H1], F8)
            nc.sync.dma_start(out=w1sb, in_=w1[:])
            w2sb = consts.tile([H1, H2], F16)
            nc.sync.dma_start(out=w2sb, in_=w2[:])
            w3sb = consts.tile([H2, H1], F16)
            nc.sync.dma_start(out=w3sb, in_=w3[:])
            wssb = consts.tile([H1, 1], F16)
            nc.sync.dma_start(out=wssb, in_=ws[:])
            b1sb = consts.tile([H1, 1], F32)
            nc.sync.dma_start(out=b1sb, in_=b1[:])
            b2sb = consts.tile([H2, 1], F32)
            nc.sync.dma_start(out=b2sb, in_=b2[:])
            b3sb = consts.tile([H1, 1], F32)
            nc.sync.dma_start(out=b3sb, in_=b3[:])
            bssb = consts.tile([1, 1], F32)
            nc.sync.dma_start(out=bssb, in_=bs[:])

            sall = opool.tile([1, NSH], F32)

            # [2048, NSH] viewed as [128 partitions, 16 chunks, NSH]
            xt_v = xt[:].rearrange("(c p) n -> p c n", p=128)

            # Software pipeline: layer k of subblock j runs alongside
            # layer 1 of subblock j+k-1, so the PE never stalls on the
            # serial act chain and stays HAM-warm.
            xtiles = {}
            h1s, h2s, h3s = {}, {}, {}
            for j in range(JTOT + 3):
                if j < JTOT:
                    i, jj = divmod(j, SPB)
                    if jj == 0:
                        xtile = xpool.tile([128, NCH, BLOCK], F8, tag="x")
                        xtiles[i] = xtile
                        if i == 0:
                            # split first load so compute starts sooner
                            for hh in range(2):
                                nc.sync.dma_start(
                                    out=xtile[:, :, hh * 1000:(hh + 1) * 1000],
                                    in_=xt_v[:, :, hh * 1000:(hh + 1) * 1000],
                                )
                        else:
                            nc.sync.dma_start(
                                out=xtile,
                                in_=xt_v[:, :, i * BLOCK:(i + 1) * BLOCK],
                            )
                    xtile = xtiles[i]
                    ps1 = pspool.tile([H1, SUB], F32, tag="ps1", bufs=3)
                    for c in range(0, NCH, 2):
                        nc.tensor.matmul(
                            ps1,
                            w1sb[:, c:c + 2, :],
                            xtile[:, c:c + 2, jj * SUB:(jj + 1) * SUB],
                            start=(c == 0),
                            stop=(c == NCH - 2),
                            perf_mode=mybir.MatmulPerfMode.DoubleRow,
                        )
                if j - 1 >= 0 and j - 1 < JTOT:
                    ps2 = pspool.tile([H2, SUB], F32, tag="ps2")
                    nc.tensor.matmul(ps2, w2sb, h1s[j - 1])
                if j - 2 >= 0 and j - 2 < JTOT:
                    ps3 = pspool.tile([H1, SUB], F32, tag="ps3")
                    nc.tensor.matmul(ps3, w3sb, h2s[j - 2])
                if j - 3 >= 0 and j - 3 < JTOT:
                    ps4 = pspool.tile([1, SUB], F32, tag="ps4", bufs=1)
                    nc.tensor.matmul(ps4, wssb, h3s[j - 3])

                if j < JTOT:
                    h1 = hpool.tile([H1, SUB], F16, tag="h1")
                    nc.scalar.activation(h1, ps1, relu, bias=b1sb, scale=1.0)
                    h1s[j] = h1
                if j - 1 >= 0 and j - 1 < JTOT:
                    h2 = hpool.tile([H2, SUB], F16, tag="h2")
                    nc.vector.tensor_scalar(
                        h2, ps2, b2sb, 0.0,
                        mybir.AluOpType.add, mybir.AluOpType.max,
                    )
                    h2s[j - 1] = h2
                if j - 2 >= 0 and j - 2 < JTOT:
                    h3 = hpool.tile([H1, SUB], F16, tag="h3")
                    nc.scalar.activation(h3, ps3, relu, bias=b3sb, scale=1.0)
                    h3s[j - 2] = h3
                if j - 3 >= 0 and j - 3 < JTOT:
                    off = (j - 3) * SUB
                    nc.vector.tensor_scalar(
                        sall[:, off:off + SUB], ps4, bssb, 0.0,
                        mybir.AluOpType.add, mybir.AluOpType.max,
                    )

            nc.sync.dma_start(out=so[:], in_=sall)

    _split_multi_waits(nc)
    return nc


def _fold_bn(w, b, g, beta):
    """Fold eval-mode BN (running mean 0, var 1) into weight/bias."""
    scale = g / np.sqrt(np.float32(1.0) + np.float32(EPS))
    return (scale[:, None] * w).astype(np.float32), (scale * b + beta).astype(
        np.float32
    )


def _exact_columns(xcols, W1p, c1, W2p, c2, W3p, c3, Wsp, cs):
    """Exact fp32 forward for a set of columns.  xcols: [M, 2048].
    Returns s [M], h3 [M, 32]."""
    h = np.maximum(xcols @ W1p.T + c1, 0.0)
    h = np.maximum(h @ W2p.T + c2, 0.0)
    h = np.maximum(h @ W3p.T + c3, 0.0)
    s = np.maximum(h @ Wsp.T + cs, 0.0)
    return s[:, 0], h


def kernel(x, W1, b1, g1, be1, W2, b2, g2, be2, W3, b3, g3, be3,
           Ws, bs, gs, bes, Wf1, bf1, gf1, bef1, Wf2, bf2, gf2, bef2,
           Wf3, bf3):
    x = np.asarray(x, dtype=np.float32)

    W1p, c1 = _fold_bn(np.asarray(W1, np.float32), np.asarray(b1, np.float32),
                       np.asarray(g1, np.float32), np.asarray(be1, np.float32))
    W2p, c2 = _fold_bn(np.asarray(W2, np.float32), np.asarray(b2, np.float32),
                       np.asarray(g2, np.float32), np.asarray(be2, np.float32))
    W3p, c3 = _fold_bn(np.asarray(W3, np.float32), np.asarray(b3, np.float32),
                       np.asarray(g3, np.float32), np.asarray(be3, np.float32))
    Wsp, cs = _fold_bn(np.asarray(Ws, np.float32), np.asarray(bs, np.float32),
                       np.asarray(gs, np.float32), np.asarray(bes, np.float32))

    # lhsT layouts: w1 [128, 16, 32] with w1[p, c, o] = W1p[o, c*128 + p]
    w1t = np.ascontiguousarray(
        W1p.T.reshape(NCH, 128, H1).transpose(1, 0, 2)
    )
    w2t = np.ascontiguousarray(W2p.T)         # [32, 8]
    w3t = np.ascontiguousarray(W3p.T)         # [8, 32]
    wst = np.ascontiguousarray(Wsp.T)         # [32, 1]

    if "nc" not in _CACHE:
        _CACHE["nc"] = _build_nc()
    nc = _CACHE["nc"]

    F8NP = mybir.dt.np(F8)
    common = {
        "w1": w1t.astype(F8NP), "w2": w2t.astype(np.float16),
        "w3": w3t.astype(np.float16), "ws": wst.astype(np.float16),
        "b1": c1.reshape(H1, 1), "b2": c2.reshape(H2, 1),
        "b3": c3.reshape(H1, 1), "bs": cs.reshape(1, 1),
    }
    in_maps = []
    for core in range(NCORES):
        b_idx, half = divmod(core, 2)
        shard = np.ascontiguousarray(
            x[b_idx, half * NSH:(half + 1) * NSH, :].T.astype(F8NP)
        )
        in_maps.append({"xt": shard, **common})

    results = run_bass_kernel_spmd(nc, in_maps, list(range(NCORES))).results

    # ---- host: safe candidate bands + exact recompute + classifier ----
    scale_f1 = (np.asarray(gf1, np.float32)
                / np.sqrt(np.float32(1.0) + np.float32(EPS)))
    scale_f2 = (np.asarray(gf2, np.float32)
                / np.sqrt(np.float32(1.0) + np.float32(EPS)))

    out = np.empty(B, dtype=np.float32)
    for b_idx in range(B):
        s_apx = np.concatenate(
            [results[2 * b_idx]["s"][0], results[2 * b_idx + 1]["s"][0]]
        ).astype(np.float32)                  # [20000] approximate scores

        def ex(cols):
            return _exact_columns(
                x[b_idx, cols, :], W1p, c1, W2p, c2, W3p, c3, Wsp, cs
            )

        # empirical device-error scale from a spread-out sample of columns
        sample = np.arange(0, N, N // 512)
        s_smp, _ = ex(sample)
        err_smp = float(np.abs(s_smp - s_apx[sample]).max())

        # initial band: generous multiple of the observed + prior error scale
        band = np.float32(max(8 * err_smp, 0.01 * float(s_apx.std()), 1e-4))
        srt = np.sort(s_apx)
        q_bot, q_top = srt[K - 1], srt[-K]

        for _attempt in range(6):
            # top band: few columns, compute all
            top_cand = np.flatnonzero(s_apx >= q_top - 2 * band)
            s_top, h_top = ex(top_cand)
            # bottom band: scan in index order, stop once K exact zeros
            # are confirmed (later candidates have s>=0 and larger index,
            # so they cannot displace earlier zeros)
            bot_cand = np.flatnonzero(s_apx <= q_bot + 2 * band)
            parts_i, parts_s, parts_h = [], [], []
            zeros = 0
            for i0 in range(0, len(bot_cand), 1024):
                ch = bot_cand[i0:i0 + 1024]
                s_c, h_c = ex(ch)
                parts_i.append(ch)
                parts_s.append(s_c)
                parts_h.append(h_c)
                zeros += int((s_c == 0.0).sum())
                if zeros >= K:
                    break
            bot_proc = np.concatenate(parts_i)
            s_bot = np.concatenate(parts_s)
            h_bot = np.concatenate(parts_h)

            err = max(
                float(np.abs(s_top - s_apx[top_cand]).max()),
                float(np.abs(s_bot - s_apx[bot_proc]).max()),
                err_smp,
            )
            if err * 4 <= band:
                break
            band = np.float32(err * 16)

        # exact stable selection (columns outside the bands provably
        # cannot reach bottom-K / top-K)
        bord = np.lexsort((bot_proc, s_bot))  # (value, index) ascending
        bot = bord[:K]
        tord = np.lexsort((top_cand, s_top))
        top = tord[-K:]

        sg = np.concatenate([s_bot[bot], s_top[top]])           # [2K]
        hsel = np.concatenate([h_bot[bot], h_top[top]]).T       # [32, 2K]
        avg = hsel.mean(axis=1)               # [32]
        feat = np.concatenate([sg, avg, hsel.reshape(-1)]).astype(np.float32)

        z = feat @ np.asarray(Wf1, np.float32).T + np.asarray(bf1, np.float32)
        z = np.maximum(z * scale_f1 + np.asarray(bef1, np.float32), 0.0)
        z = z @ np.asarray(Wf2, np.float32).T + np.asarray(bf2, np.float32)
        z = np.maximum(z * scale_f2 + np.asarray(bef2, np.float32), 0.0)
        logit = z @ np.asarray(Wf3, np.float32).T + np.asarray(bf3, np.float32)
        out[b_idx] = 1.0 / (1.0 + np.exp(-logit[0]))

    return out


# revision 19
# speedup vs baseline: 1.0069x; 1.0069x over previous
"""Trainium2 Bass kernel: pointnet-style conv stack + score head + top/bottom-K
selection + tiny classifier.

Pipeline (per batch b of 4):
  xT = x[b].T                      [2048, 20000]
  h  = relu(bn(conv 2048->32->8->32))   (conv1d k=1 == matmul over channels)
  s  = relu(bn(conv 32->1))        scores [20000]
  sel = bottom-10 + top-10 indices of stable-ascending argsort(s)
  feat = [s[sel], mean(h[:, sel], -1), h[:, sel].flat]  (692)
  out[b] = sigmoid(classifier(feat))

Strategy:
  * 8 cores = 4 batches x 2 N-halves; each core gets an x.T shard
    [2048, 10000] (host-transposed so the contraction dim D lands on SBUF
    partitions).  The kernel is memory-bound on reading x (82 MB/core).
  * The device computes APPROXIMATE scores s for every column: layer 1
    runs in fp8e4m3 (host-cast x; quarter DMA bytes) with DoubleRow
    matmuls (K=256 per pass), the tiny later layers in fp16.  The
    small layers are software-pipelined 1-3 subblocks behind layer 1
    so the PE never stalls on the activation chain.
  * The host then takes a provably-safe candidate band around the
    bottom-10/top-10 of the approximate scores, recomputes those few
    columns exactly in fp32, and does the exact selection + tiny
    classifier.  Final output error does not depend on device precision
    as long as the band covers the device's score error (band width is
    validated against the observed error and widened if needed).
"""

import numpy as np

import concourse.bass as bass
import concourse.mybir as mybir
import concourse.tile as tile
from concourse.bass_utils import run_bass_kernel_spmd

F32 = mybir.dt.float32
F16 = mybir.dt.float16
F8 = mybir.dt.float8e4

B = 4
N = 20000
D = 2048
H1 = 32
H2 = 8
K = 10
EPS = 1e-5
NCORES = 8
NSH = N // 2           # 10000 columns per core shard
BLOCK = 2000           # columns of x per DMA (4.2 MB fp8 per transfer)
SUB = 500              # matmul moving free dim (<= 512 for 4-byte dtypes)
NCH = D // 128         # 16 contraction chunks of 128

_CACHE = {}


def _split_multi_waits(nc):
    """Walrus in this container only encodes ONE sync wait per instruction
    ("Too many sync wait commands").  Tile attaches several (PE sem + DMA
    lane sems...).  Hoist all-but-one wait onto standalone InstEventSemaphore
    instructions on the same engine queue right before the instruction —
    engine queues are in-order, so semantics are preserved."""
    wid = 0
    for f in nc.m.functions:
        for blk in f.blocks:
            insts = blk.instructions
            for idx in range(len(insts) - 1, -1, -1):
                inst = insts[idx]
                si = inst.sync_info
                if si is None or len(si.on_wait) <= 1:
                    continue
                waits = list(si.on_wait)
                inst.sync_info = mybir.SyncInfo(
                    on_wait=[waits[-1]], on_update=list(si.on_update)
                )
                for w in reversed(waits[:-1]):
                    wid += 1
                    ev = mybir.InstEventSemaphore(
                        name=f"WSPLIT-{wid}", ins=[], outs=[]
                    )
                    ev.engine = inst.engine
                    ev.sync_info = mybir.SyncInfo(on_wait=[w], on_update=[])
                    insts.insert(idx, ev)


def _build_nc():
    nc = bass.Bass()
    xt = nc.declare_dram_parameter("xt", [D, NSH], F8, isOutput=False)
    w1 = nc.declare_dram_parameter("w1", [128, NCH, H1], F8, isOutput=False)
    w2 = nc.declare_dram_parameter("w2", [H1, H2], F16, isOutput=False)
    w3 = nc.declare_dram_parameter("w3", [H2, H1], F16, isOutput=False)
    ws = nc.declare_dram_parameter("ws", [H1, 1], F16, isOutput=False)
    b1 = nc.declare_dram_parameter("b1", [H1, 1], F32, isOutput=False)
    b2 = nc.declare_dram_parameter("b2", [H2, 1], F32, isOutput=False)
    b3 = nc.declare_dram_parameter("b3", [H1, 1], F32, isOutput=False)
    bs = nc.declare_dram_parameter("bs", [1, 1], F32, isOutput=False)
    so = nc.declare_dram_parameter("s", [1, NSH], F32, isOutput=True)

    relu = mybir.ActivationFunctionType.Relu

    JTOT = NSH // SUB          # 20 subblocks
    SPB = BLOCK // SUB         # subblocks per x DMA block

    with tile.TileContext(nc) as tc:
        with (
            tc.tile_pool(name="consts", bufs=1) as consts,
            tc.tile_pool(name="xpool", bufs=3) as xpool,
            tc.tile_pool(name="hpool", bufs=3) as hpool,
            tc.tile_pool(name="opool", bufs=1) as opool,
            tc.tile_pool(name="pspool", bufs=2, space="PSUM") as pspool,
        ):
            w1sb = consts.tile([128, NCH, H1], F8)
            nc.sync.dma_start(out=w1sb, in_=w1[:])
            w2sb = consts.tile([H1, H2], F16)
            nc.sync.dma_start(out=w2sb, in_=w2[:])
            w3sb = consts.tile([H2, H1], F16)
            nc.sync.dma_start(out=w3sb, in_=w3[:])
            wssb = consts.tile([H1, 1], F16)
            nc.sync.dma_start(out=wssb, in_=ws[:])
            b1sb = consts.tile([H1, 1], F32)
            nc.sync.dma_start(out=b1sb, in_=b1[:])
            b2sb = consts.tile([H2, 1], F32)
            nc.sync.dma_start(out=b2sb, in_=b2[:])
            b3sb = consts.tile([H1, 1], F32)
            nc.sync.dma_start(out=b3sb, in_=b3[:])
            bssb = consts.tile([1, 1], F32)
            nc.sync.dma_start(out=bssb, in_=bs[:])

            sall = opool.tile([1, NSH], F32)

            # [2048, NSH] viewed as [128 partitions, 16 chunks, NSH]
            xt_v = xt[:].rearrange("(c p) n -> p c n", p=128)

            # Software pipeline: layer k of subblock j runs alongside
            # layer 1 of subblock j+k-1, so the PE never stalls on the
            # serial act chain and stays HAM-warm.
            xtiles = {}
            h1s, h2s, h3s = {}, {}, {}
            for j in range(JTOT + 3):
                if j < JTOT:
                    i, jj = divmod(j, SPB)
                    if jj == 0:
                        xtile = xpool.tile([128, NCH, BLOCK], F8, tag="x")
                        xtiles[i] = xtile
                        if i == 0:
                            # split first load so compute starts sooner
                            for hh in range(2):
                                nc.sync.dma_start(
                                    out=xtile[:, :, hh * 1000:(hh + 1) * 1000],
                                    in_=xt_v[:, :, hh * 1000:(hh + 1) * 1000],
                                )
                        else:
                            nc.sync.dma_start(
                                out=xtile,
                                in_=xt_v[:, :, i * BLOCK:(i + 1) * BLOCK],
                            )
                    xtile = xtiles[i]
                    ps1 = pspool.tile([H1, SUB], F32, tag="ps1", bufs=3)
                    for c in range(0, NCH, 2):
                        nc.tensor.matmul(
                            ps1,
                            w1sb[:, c:c + 2, :],
                            xtile[:, c:c + 2, jj * SUB:(jj + 1) * SUB],
                            start=(c == 0),
                            stop=(c == NCH - 2),
                            perf_mode=mybir.MatmulPerfMode.DoubleRow,
                        )
                if j - 1 >= 0 and j - 1 < JTOT:
                    ps2 = pspool.tile([H2, SUB], F32, tag="ps2")
                    nc.tensor.matmul(ps2, w2sb, h1s[j - 1])
                if j - 2 >= 0 and j - 2 < JTOT:
                    ps3 = pspool.tile([H1, SUB], F32, tag="ps3")
                    nc.tensor.matmul(ps3, w3sb, h2s[j - 2])
                if j - 3 >= 0 and j - 3 < JTOT:
                    ps4 = pspool.tile([1, SUB], F32, tag="ps4", bufs=1)
                    nc.tensor.matmul(ps4, wssb, h3s[j - 3])

                if j < JTOT:
                    h1 = hpool.tile([H1, SUB], F16, tag="h1")
                    nc.scalar.activation(h1, ps1, relu, bias=b1sb, scale=1.0)
                    h1s[j] = h1
                if j - 1 >= 0 and j - 1 < JTOT:
                    h2 = hpool.tile([H2, SUB], F16, tag="h2")
                    nc.vector.tensor_scalar(
                        h2, ps2, b2sb, 0.0,
                        mybir.AluOpType.add, mybir.AluOpType.max,
                    )
                    h2s[j - 1] = h2
                if j - 2 >= 0 and j - 2 < JTOT:
                    h3 = hpool.tile([H1, SUB], F16, tag="h3")
                    nc.scalar.activation(h3, ps3, relu, bias=b3sb, scale=1.0)
                    h3s[j - 2] = h3
                if j - 3 >= 0 and j - 3 < JTOT:
                    off = (j - 3) * SUB
                    nc.vector.tensor_scalar(
                        sall[:, off:off + SUB], ps4, bssb, 0.0,
                        mybir.AluOpType.add, mybir.AluOpType.max,
                    )

            nc.sync.dma_start(out=so[:], in_=sall)

    _split_multi_waits(nc)
    return nc


def _fold_bn(w, b, g, beta):
    """Fold eval-mode BN (running mean 0, var 1) into weight/bias."""
    scale = g / np.sqrt(np.float32(1.0) + np.float32(EPS))
    return (scale[:, None] * w).astype(np.float32), (scale * b + beta).astype(
        np.float32
    )


def _exact_columns(xcols, W1p, c1, W2p, c2, W3p, c3, Wsp, cs):
    """Exact fp32 forward for a set of columns.  xcols: [M, 2048].
    Returns s [M], h3 [M, 32]."""
    h = np.maximum(xcols @ W1p.T + c1, 0.0)
    h = np.maximum(h @ W2p.T + c2, 0.0)
    h = np.maximum(h @ W3p.T + c3, 0.0)
    s = np.maximum(h @ Wsp.T + cs, 0.0)
    return s[:, 0], h


def kernel(x, W1, b1, g1, be1, W2, b2, g2, be2, W3, b3, g3, be3,
           Ws, bs, gs, bes, Wf1, bf1, gf1, bef1, Wf2, bf2, gf2, bef2,
           Wf3, bf3):
    x = np.asarray(x, dtype=np.float32)

    W1p, c1 = _fold_bn(np.asarray(W1, np.float32), np.asarray(b1, np.float32),
                       np.asarray(g1, np.float32), np.asarray(be1, np.float32))
    W2p, c2 = _fold_bn(np.asarray(W2, np.float32), np.asarray(b2, np.float32),
                       np.asarray(g2, np.float32), np.asarray(be2, np.float32))
    W3p, c3 = _fold_bn(np.asarray(W3, np.float32), np.asarray(b3, np.float32),
                       np.asarray(g3, np.float32), np.asarray(be3, np.float32))
    Wsp, cs = _fold_bn(np.asarray(Ws, np.float32), np.asarray(bs, np.float32),
                       np.asarray(gs, np.float32), np.asarray(bes, np.float32))

    # lhsT layouts: w1 [128, 16, 32] with w1[p, c, o] = W1p[o, c*128 + p]
    w1t = np.ascontiguousarray(
        W1p.T.reshape(NCH, 128, H1).transpose(1, 0, 2)
    )
    w2t = np.ascontiguousarray(W2p.T)         # [32, 8]
    w3t = np.ascontiguousarray(W3p.T)         # [8, 32]
    wst = np.ascontiguousarray(Wsp.T)         # [32, 1]

    if "nc" not in _CACHE:
        _CACHE["nc"] = _build_nc()
    nc = _CACHE["nc"]

    F8NP = mybir.dt.np(F8)
    common = {
        "w1": w1t.astype(F8NP), "w2": w2t.astype(np.float16),
        "w3": w3t.astype(np.float16), "ws": wst.astype(np.float16),
        "b1": c1.reshape(H1, 1), "b2": c2.reshape(H2, 1),
        "b3": c3.reshape(H1, 1), "bs": cs.reshape(1, 1),
    }
    in_maps = []
    for core in range(NCORES):
        b_idx, half = divmod(core, 2)
        shard = np.ascontiguousarray(
            x[b_idx, half * NSH:(half + 1) * NSH, :].T.astype(F8NP)
        )
        in_maps.append({"xt": shard, **common})

    results = run_bass_kernel_spmd(nc, in_maps, list(range(NCORES))).results

    # ---- host: safe candidate bands + exact recompute + classifier ----
    scale_f1 = (np.asarray(gf1, np.float32)
                / np.sqrt(np.float32(1.0) + np.float32(EPS)))
    scale_f2 = (np.asarray(gf2, np.float32)
                / np.sqrt(np.float32(1.0) + np.float32(EPS)))

    out = np.empty(B, dtype=np.float32)
    for b_idx in range(B):
        s_apx = np.concatenate(
            [results[2 * b_idx]["s"][0], results[2 * b_idx + 1]["s"][0]]
        ).astype(np.float32)                  # [20000] approximate scores

        def ex(cols):
            return _exact_columns(
                x[b_idx, cols, :], W1p, c1, W2p, c2, W3p, c3, Wsp, cs
            )

        # empirical device-error scale from a spread-out sample of columns
        sample = np.arange(0, N, N // 512)
        s_smp, _ = ex(sample)
        err_smp = float(np.abs(s_smp - s_apx[sample]).max())

        # initial band: generous multiple of the observed + prior error scale
        band = np.float32(max(8 * err_smp, 0.01 * float(s_apx.std()), 1e-4))
        srt = np.sort(s_apx)
        q_bot, q_top = srt[K - 1], srt[-K]

        for _attempt in range(6):
            # top band: few columns, compute all
            top_cand = np.flatnonzero(s_apx >= q_top - 2 * band)
            s_top, h_top = ex(top_cand)
            # bottom band: scan in index order, stop once K exact zeros
            # are confirmed (later candidates have s>=0 and larger index,
            # so they cannot displace earlier zeros)
            bot_cand = np.flatnonzero(s_apx <= q_bot + 2 * band)
            parts_i, parts_s, parts_h = [], [], []
            zeros = 0
            for i0 in range(0, len(bot_cand), 1024):
                ch = bot_cand[i0:i0 + 1024]
                s_c, h_c = ex(ch)
                parts_i.append(ch)
                parts_s.append(s_c)
                parts_h.append(h_c)
                zeros += int((s_c == 0.0).sum())
                if zeros >= K:
                    break
            bot_proc = np.concatenate(parts_i)
            s_bot = np.concatenate(parts_s)
            h_bot = np.concatenate(parts_h)

            err = max(
                float(np.abs(s_top - s_apx[top_cand]).max()),
                float(np.abs(s_bot - s_apx[bot_proc]).max()),
                err_smp,
            )
            if err * 4 <= band:
                break
            band = np.float32(err * 16)

        # exact stable selection (columns outside the bands provably
        # cannot reach bottom-K / top-K)
        bord = np.lexsort((bot_proc, s_bot))  # (value, index) ascending
        bot = bord[:K]
        tord = np.lexsort((top_cand, s_top))
        top = tord[-K:]

        sg = np.concatenate([s_bot[bot], s_top[top]])           # [2K]
        hsel = np.concatenate([h_bot[bot], h_top[top]]).T       # [32, 2K]
        avg = hsel.mean(axis=1)               # [32]
        feat = np.concatenate([sg, avg, hsel.reshape(-1)]).astype(np.float32)

        z = feat @ np.asarray(Wf1, np.float32).T + np.asarray(bf1, np.float32)
        z = np.maximum(z * scale_f1 + np.asarray(bef1, np.float32), 0.0)
        z = z @ np.asarray(Wf2, np.float32).T + np.asarray(bf2, np.float32)
        z = np.maximum(z * scale_f2 + np.asarray(bef2, np.float32), 0.0)
        logit = z @ np.asarray(Wf3, np.float32).T + np.asarray(bf3, np.float32)
        out[b_idx] = 1.0 / (1.0 + np.exp(-logit[0]))

    return out
